# revision 8
# baseline (speedup 1.0000x reference)
"""Greedy autoregressive LSTM decoder on 8 trn2 NeuronCores.

Strategy: vocab-shard the big projection out_W (32000x512) across 8 cores
(4000 rows each, SBUF-resident). Every core runs the full-batch (B=64) LSTM
recurrence redundantly (cheap). Each step, cores compute their local logits
slice, local (max, sumexp, argmax) stats, AllGather the tiny stats vector,
combine locally to get the global log-softmax normalizer and global argmax,
write their logp slice, and gather the next embedding row.

Host pre-fuses relu(emb[qix_to_aix]) into one (32000, 300) gather table so
the device does a single indirect DMA per step. The LSTM bias rides in the
zero-padding rows of the W_ih K-tiles (x is padded with a ones column).
sumexp is computed without max-shift: logits here are O(1) so exp cannot
overflow, and that makes the cross-core combine a plain sum.
"""

import numpy as np

B = 64
H = 512
E = 300
EP = 384  # E (+ ones row at 300) padded to 3*128
G = 2048  # 4*H
VQ = 32000
NCORES = 8
VS = VQ // NCORES  # 4000
VSP = 4096  # padded vocab slice
NCH = 8  # vocab chunks of 512
SOS = 1
NEG_BIG = -1.0e30

_cache = {}


def _build(T1):
    import concourse.bass as bass
    import concourse.bacc as bacc
    import concourse.tile as tile
    import concourse.mybir as mybir

    f32 = mybir.dt.float32
    i32 = mybir.dt.int32
    AF = mybir.ActivationFunctionType
    OP = mybir.AluOpType
    AX = mybir.AxisListType

    nc = bacc.Bacc(
        "TRN2",
        target_bir_lowering=False,
        debug=False,
        enable_asserts=False,
        num_devices=NCORES,
    )

    X0T = nc.dram_tensor("x0t", [128, 3, B], f32, kind="ExternalInput")
    H0T = nc.dram_tensor("h0t", [128, 4, B], f32, kind="ExternalInput")
    WIH = nc.dram_tensor("wih", [128, 3, G], f32, kind="ExternalInput")
    WHH = nc.dram_tensor("whh", [128, 4, G], f32, kind="ExternalInput")
    OUTW = nc.dram_tensor("outw", [128, 4, VSP], f32, kind="ExternalInput")
    OUTB = nc.dram_tensor("outb", [B, VSP], f32, kind="ExternalInput")
    IOTA0 = nc.dram_tensor("iota0", [B, 512], f32, kind="ExternalInput")
    CHOF = nc.dram_tensor("chof", [B, NCH], f32, kind="ExternalInput")
    EMBX = nc.dram_tensor("embx", [VQ, E], f32, kind="ExternalInput")
    IDENT = nc.dram_tensor("identm", [B, B], f32, kind="ExternalInput")
    OUT = nc.dram_tensor("out", [B, T1, VS], f32, kind="ExternalOutput")

    with tile.TileContext(nc) as tc:
        with (
            tc.tile_pool(name="const", bufs=1) as constp,
            tc.tile_pool(name="state", bufs=1) as statep,
            tc.tile_pool(name="lstm", bufs=1) as lstmp,
            tc.tile_pool(name="logits", bufs=2) as logitsp,
            tc.tile_pool(name="chunk", bufs=2) as chunkp,
            tc.tile_pool(name="work", bufs=2) as workp,
            tc.tile_pool(name="psum", bufs=2, space="PSUM") as psump,
            tc.tile_pool(name="psg", bufs=2, space="PSUM") as psgp,
            tc.tile_pool(name="dram", bufs=2, space="DRAM") as dramp,
        ):
            # ---- constants (loaded once) ----
            wih = constp.tile([128, 3, G], f32)
            whh = constp.tile([128, 4, G], f32)
            outw = constp.tile([128, 4, VSP], f32)
            outb = constp.tile([B, VSP], f32)
            iota0 = constp.tile([B, 512], f32)
            chof = constp.tile([B, NCH], f32)
            ident = constp.tile([B, B], f32)
            nc.sync.dma_start(wih[:], WIH.ap())
            nc.sync.dma_start(whh[:], WHH.ap())
            nc.sync.dma_start(outw[:], OUTW.ap())
            nc.sync.dma_start(outb[:], OUTB.ap())
            nc.sync.dma_start(iota0[:], IOTA0.ap())
            nc.sync.dma_start(chof[:], CHOF.ap())
            nc.sync.dma_start(ident[:], IDENT.ap())

            # ---- persistent state ----
            xT = statep.tile([128, 3, B], f32)  # x^T (K on partitions)
            hT = statep.tile([128, 4, B], f32)
            cst = statep.tile([B, H], f32)
            xpad = statep.tile([B, EP], f32)  # col 300 = 1.0 (bias row)
            nc.sync.dma_start(xT[:], X0T.ap())
            nc.sync.dma_start(hT[:], H0T.ap())
            nc.vector.memset(cst[:], 0.0)
            nc.vector.memset(xpad[:], 0.0)
            nc.vector.memset(xpad[:, E:E + 1], 1.0)

            for t in range(T1):
                last = t == T1 - 1
                # ---- gates = x @ W_ih.T + h @ W_hh.T + b (b in wih row E) ----
                ig = lstmp.tile([B, H], f32, tag="ig")
                fg = lstmp.tile([B, H], f32, tag="fg")
                gg = lstmp.tile([B, H], f32, tag="gg")
                og = lstmp.tile([B, H], f32, tag="og")
                gact = [(ig, AF.Sigmoid), (fg, AF.Sigmoid),
                        (gg, AF.Tanh), (og, AF.Sigmoid)]
                for ch in range(4):
                    pg = psgp.tile([B, 512], f32, tag="pg")
                    sl = slice(ch * 512, (ch + 1) * 512)
                    for k in range(3):
                        nc.tensor.matmul(
                            pg[:], xT[:, k, :], wih[:, k, sl],
                            start=(k == 0), stop=False,
                        )
                    for k in range(4):
                        nc.tensor.matmul(
                            pg[:], hT[:, k, :], whh[:, k, sl],
                            start=False, stop=(k == 3),
                        )
                    gt, fn = gact[ch]
                    nc.scalar.activation(gt[:], pg[:], fn)
                # ---- LSTM cell state update ----
                nc.vector.tensor_tensor(ig[:], ig[:], gg[:], op=OP.mult)
                nc.vector.tensor_tensor(cst[:], fg[:], cst[:], op=OP.mult)
                nc.vector.tensor_tensor(cst[:], cst[:], ig[:], op=OP.add)
                nc.scalar.activation(fg[:], cst[:], AF.Tanh)
                nc.vector.tensor_tensor(gg[:], og[:], fg[:], op=OP.mult)
                hh = gg  # new hidden state, batch-partition layout
                # ---- transpose h -> hT ----
                for k in range(4):
                    pt = psump.tile([128, B], f32, tag="pt")
                    nc.tensor.transpose(
                        pt[:], hh[:, k * 128:(k + 1) * 128], ident[:]
                    )
                    nc.scalar.activation(hT[:, k, :], pt[:], AF.Copy)
                # ---- vocab projection; per-chunk bias/max/exp/argmax ----
                logits = logitsp.tile([B, VSP], f32, tag="logits")
                cmax = workp.tile([B, NCH], f32, tag="cmax")
                csum = workp.tile([B, NCH], f32, tag="csum")
                cidx = workp.tile([B, NCH], f32, tag="cidx")
                for ch in range(NCH):
                    pv = psump.tile([B, 512], f32, tag="pv")
                    sl = slice(ch * 512, (ch + 1) * 512)
                    for k in range(4):
                        nc.tensor.matmul(
                            pv[:], hT[:, k, :], outw[:, k, sl],
                            start=(k == 0), stop=(k == 3),
                        )
                    # logits = pv + outb, then chunk max
                    nc.vector.tensor_tensor(
                        out=logits[:, sl], in0=pv[:], in1=outb[:, sl],
                        op=OP.add,
                    )
                    nc.vector.tensor_reduce(
                        out=cmax[:, ch:ch + 1], in_=logits[:, sl],
                        op=OP.max, axis=AX.X,
                    )
                    # sumexp of chunk (no max-shift; logits are O(1))
                    scr = chunkp.tile([B, 512], f32, tag="scr")
                    nc.scalar.activation(
                        out=scr[:], in_=logits[:, sl], func=AF.Exp,
                        accum_out=csum[:, ch:ch + 1],
                    )
                    # argmax-in-chunk: sum((logits >= cmax) * iota), one pass
                    jnk = chunkp.tile([B, 512], f32, tag="jnk")
                    nc.vector.scalar_tensor_tensor(
                        out=jnk[:], in0=logits[:, sl],
                        scalar=cmax[:, ch:ch + 1], in1=iota0[:],
                        op0=OP.is_ge, op1=OP.mult,
                        accum_out=cidx[:, ch:ch + 1],
                    )
                # ---- local stats -> [max, sumexp, globalidx] ----
                stats = workp.tile([B, 3], f32, tag="stats")
                nc.vector.tensor_reduce(
                    out=stats[:, 0:1], in_=cmax[:], op=OP.max, axis=AX.X
                )
                nc.vector.tensor_reduce(
                    out=stats[:, 1:2], in_=csum[:], op=OP.add, axis=AX.X
                )
                gidx8 = workp.tile([B, NCH], f32, tag="gidx8")
                nc.vector.tensor_tensor(
                    gidx8[:], cidx[:], chof[:], op=OP.add
                )
                jnk8 = workp.tile([B, NCH], f32, tag="jnk8")
                nc.vector.scalar_tensor_tensor(
                    out=jnk8[:], in0=cmax[:], scalar=stats[:, 0:1],
                    in1=gidx8[:], op0=OP.is_ge, op1=OP.mult,
                    accum_out=stats[:, 2:3],
                )
                # ---- AllGather stats ----
                sdram = dramp.tile([B, 3], f32, tag="sin")
                gdram = dramp.tile([NCORES * B, 3], f32, tag="gout")
                nc.gpsimd.dma_start(sdram[:], stats[:])
                nc.gpsimd.collective_compute(
                    "AllGather",
                    OP.bypass,
                    ins=[sdram[:]],
                    outs=[gdram[:]],
                    replica_groups=[list(range(NCORES))],
                )
                gath = workp.tile([B, NCORES, 3], f32, tag="gath")
                nc.gpsimd.dma_start(
                    gath[:], gdram[:].rearrange("(r b) s -> b r s", r=NCORES)
                )
                # ---- combine: logZ = ln(sum of sumexps) ----
                gsum = workp.tile([B, 1], f32, tag="gsum")
                nc.vector.tensor_reduce(
                    out=gsum[:], in_=gath[:, :, 1], op=OP.add, axis=AX.X
                )
                lngs = workp.tile([B, 1], f32, tag="lngs")
                nc.scalar.activation(lngs[:], gsum[:], AF.Ln)
                nlz = workp.tile([B, 1], f32, tag="nlz")
                nc.vector.tensor_scalar_mul(nlz[:], lngs[:], -1.0)
                # ---- logp slice -> DRAM (ACT identity with bias, in place) ----
                nc.scalar.activation(
                    out=logits[:], in_=logits[:], func=AF.Identity,
                    bias=nlz[:, 0:1],
                )
                for q in range(4):
                    nc.sync.dma_start(
                        OUT.ap()[:, t, q * 1000:(q + 1) * 1000],
                        logits[:, q * 1000:(q + 1) * 1000],
                    )
                if last:
                    continue
                # ---- global argmax index + next-x gather ----
                gmax = workp.tile([B, 1], f32, tag="gmax")
                nc.vector.tensor_reduce(
                    out=gmax[:], in_=gath[:, :, 0], op=OP.max, axis=AX.X
                )
                jnkr = workp.tile([B, NCORES], f32, tag="jnkr")
                gidx = workp.tile([B, 1], f32, tag="gidx")
                nc.vector.scalar_tensor_tensor(
                    out=jnkr[:], in0=gath[:, :, 0], scalar=gmax[:, 0:1],
                    in1=gath[:, :, 2], op0=OP.is_ge, op1=OP.mult,
                    accum_out=gidx[:],
                )
                nc.vector.tensor_scalar(
                    out=gidx[:], in0=gidx[:], scalar1=float(VQ - 1),
                    scalar2=0.0, op0=OP.min, op1=OP.max,
                )
                idxi = workp.tile([B, 1], i32, tag="idxi")
                nc.vector.tensor_copy(idxi[:], gidx[:])
                nc.gpsimd.indirect_dma_start(
                    out=xpad[:, 0:E],
                    out_offset=None,
                    in_=EMBX.ap(),
                    in_offset=bass.IndirectOffsetOnAxis(ap=idxi[:, 0:1], axis=0),
                )
                for k in range(3):
                    pt = psump.tile([128, B], f32, tag="pt")
                    nc.tensor.transpose(
                        pt[:], xpad[:, k * 128:(k + 1) * 128], ident[:]
                    )
                    nc.scalar.activation(xT[:, k, :], pt[:], AF.Copy)

    nc.finalize()
    return nc


def _prep_inputs(input_h, q_att, emb, W_ih, W_hh, b_ih, b_hh, out_W, out_b,
                 qix_to_aix):
    embx = np.maximum(
        np.asarray(emb, np.float32)[np.asarray(qix_to_aix, np.int64)], 0.0
    ).astype(np.float32)
    embx = np.ascontiguousarray(embx)
    x0 = embx[SOS]  # (300,)
    x0t = np.zeros((EP, B), np.float32)
    x0t[:E, :] = x0[:, None]
    x0t[E, :] = 1.0  # ones row driving the fused bias
    x0t = np.ascontiguousarray(x0t.reshape(3, 128, B).transpose(1, 0, 2))
    h0t = np.ascontiguousarray(
        np.asarray(q_att, np.float32).T.reshape(4, 128, B).transpose(1, 0, 2)
    )
    wih = np.zeros((EP, G), np.float32)
    wih[:E, :] = np.asarray(W_ih, np.float32).T
    wih[E, :] = np.asarray(b_ih, np.float32) + np.asarray(b_hh, np.float32)
    wih = np.ascontiguousarray(wih.reshape(3, 128, G).transpose(1, 0, 2))
    whh = np.ascontiguousarray(
        np.asarray(W_hh, np.float32).T.reshape(4, 128, G).transpose(1, 0, 2)
    )
    iota0 = np.ascontiguousarray(
        np.broadcast_to(np.arange(512, dtype=np.float32), (B, 512))
    )
    identm = np.ascontiguousarray(np.eye(B, dtype=np.float32))
    shared = dict(x0t=x0t, h0t=h0t, wih=wih, whh=whh, iota0=iota0, embx=embx,
                  identm=identm)
    in_maps = []
    for i in range(NCORES):
        sl = slice(i * VS, (i + 1) * VS)
        ow = np.zeros((H, VSP), np.float32)
        ow[:, :VS] = np.asarray(out_W, np.float32)[sl].T
        ow = np.ascontiguousarray(ow.reshape(4, 128, VSP).transpose(1, 0, 2))
        ob = np.full((VSP,), NEG_BIG, np.float32)
        ob[:VS] = np.asarray(out_b, np.float32)[sl]
        obr = np.ascontiguousarray(np.broadcast_to(ob, (B, VSP)))
        co = (i * VS + np.arange(NCH, dtype=np.float32) * 512)
        cor = np.ascontiguousarray(np.broadcast_to(co, (B, NCH)))
        m = dict(shared)
        m.update(outw=ow, outb=obr, chof=cor)
        in_maps.append(m)
    return in_maps


def kernel(input_h, q_att, emb, W_ih, W_hh, b_ih, b_hh, out_W, out_b,
           qix_to_aix, max_len, _want_results=False, _run_kwargs=None):
    from concourse import bass_utils

    T1 = int(max_len) + 1
    if T1 not in _cache:
        _cache[T1] = _build(T1)
    nc = _cache[T1]
    in_maps = _prep_inputs(input_h, q_att, emb, W_ih, W_hh, b_ih, b_hh,
                           out_W, out_b, qix_to_aix)
    res = bass_utils.run_bass_kernel_spmd(
        nc, in_maps, core_ids=list(range(NCORES)), **(_run_kwargs or {})
    )
    out = np.concatenate([res.results[i]["out"] for i in range(NCORES)],
                         axis=2)
    if _want_results:
        return out, res
    return out


# revision 9
# speedup vs baseline: 1.5499x; 1.5499x over previous
"""Greedy autoregressive LSTM decoder on 8 trn2 NeuronCores.

Strategy: vocab-shard the big projection out_W (32000x512) across 8 cores
(4000 rows each, SBUF-resident). Every core runs the full-batch (B=64) LSTM
recurrence redundantly (cheap). Each step, cores compute their local logits
slice, local (max, sumexp, argmax) stats, AllGather the tiny stats vector,
combine locally to get the global log-softmax normalizer and global argmax,
write their logp slice, and gather the next embedding row.

Host pre-fuses relu(emb[qix_to_aix]) into one (32000, 300) gather table so
the device does a single indirect DMA per step. The LSTM bias rides in the
zero-padding rows of the W_ih K-tiles (x is padded with a ones column).
sumexp is computed without max-shift: logits here are O(1) so exp cannot
overflow, and that makes the cross-core combine a plain sum.
"""

import numpy as np

B = 64
H = 512
E = 300
EP = 384  # E (+ ones row at 300) padded to 3*128
G = 2048  # 4*H
VQ = 32000
NCORES = 8
VS = VQ // NCORES  # 4000
VSP = 4096  # padded vocab slice
NCH = 8  # vocab chunks of 512
SOS = 1
NEG_BIG = -1.0e30

_cache = {}


def _build(T1):
    import concourse.bass as bass
    import concourse.bacc as bacc
    import concourse.tile as tile
    import concourse.mybir as mybir

    f32 = mybir.dt.float32
    f32r = mybir.dt.float32r
    i32 = mybir.dt.int32
    AF = mybir.ActivationFunctionType
    OP = mybir.AluOpType
    AX = mybir.AxisListType

    nc = bacc.Bacc(
        "TRN2",
        target_bir_lowering=False,
        debug=False,
        enable_asserts=False,
        num_devices=NCORES,
    )

    X0T = nc.dram_tensor("x0t", [128, 3, B], f32r, kind="ExternalInput")
    H0T = nc.dram_tensor("h0t", [128, 4, B], f32r, kind="ExternalInput")
    WIH = nc.dram_tensor("wih", [128, 3, G], f32r, kind="ExternalInput")
    WHH = nc.dram_tensor("whh", [128, 4, G], f32r, kind="ExternalInput")
    OUTW = nc.dram_tensor("outw", [128, 4, VSP], f32r, kind="ExternalInput")
    OUTB = nc.dram_tensor("outb", [B, VSP], f32, kind="ExternalInput")
    IOTA0 = nc.dram_tensor("iota0", [B, 512], f32, kind="ExternalInput")
    CHOF = nc.dram_tensor("chof", [B, NCH], f32, kind="ExternalInput")
    EMBX = nc.dram_tensor("embx", [VQ, E], f32, kind="ExternalInput")
    IDENT = nc.dram_tensor("identm", [B, B], f32, kind="ExternalInput")
    OUT = nc.dram_tensor("out", [B, T1, VS], f32, kind="ExternalOutput")

    with tile.TileContext(nc) as tc:
        with (
            tc.tile_pool(name="const", bufs=1) as constp,
            tc.tile_pool(name="state", bufs=1) as statep,
            tc.tile_pool(name="lstm", bufs=1) as lstmp,
            tc.tile_pool(name="logits", bufs=2) as logitsp,
            tc.tile_pool(name="chunk", bufs=2) as chunkp,
            tc.tile_pool(name="work", bufs=2) as workp,
            tc.tile_pool(name="psum", bufs=2, space="PSUM") as psump,
            tc.tile_pool(name="psg", bufs=2, space="PSUM") as psgp,
            tc.tile_pool(name="dram", bufs=2, space="DRAM") as dramp,
        ):
            # ---- constants (loaded once) ----
            wih = constp.tile([128, 3, G], f32r)
            whh = constp.tile([128, 4, G], f32r)
            outw = constp.tile([128, 4, VSP], f32r)
            outb = constp.tile([B, VSP], f32)
            iota0 = constp.tile([B, 512], f32)
            chof = constp.tile([B, NCH], f32)
            ident = constp.tile([B, B], f32)
            nc.sync.dma_start(wih[:], WIH.ap())
            nc.sync.dma_start(whh[:], WHH.ap())
            nc.sync.dma_start(outw[:], OUTW.ap())
            nc.sync.dma_start(outb[:], OUTB.ap())
            nc.sync.dma_start(iota0[:], IOTA0.ap())
            nc.sync.dma_start(chof[:], CHOF.ap())
            nc.sync.dma_start(ident[:], IDENT.ap())

            # ---- persistent state ----
            xT = statep.tile([128, 3, B], f32r)  # x^T (K on partitions)
            hT = statep.tile([128, 4, B], f32r)
            cst = statep.tile([B, H], f32)
            xpad = statep.tile([B, EP], f32)  # col 300 = 1.0 (bias row)
            nc.sync.dma_start(xT[:], X0T.ap())
            nc.sync.dma_start(hT[:], H0T.ap())
            nc.vector.memset(cst[:], 0.0)
            nc.vector.memset(xpad[:], 0.0)
            nc.vector.memset(xpad[:, E:E + 1], 1.0)

            for t in range(T1):
                last = t == T1 - 1
                # ---- gates = x @ W_ih.T + h @ W_hh.T + b (b in wih row E) ----
                ig = lstmp.tile([B, H], f32, tag="ig")
                fg = lstmp.tile([B, H], f32, tag="fg")
                gg = lstmp.tile([B, H], f32, tag="gg")
                og = lstmp.tile([B, H], f32, tag="og")
                gact = [(ig, AF.Sigmoid), (fg, AF.Sigmoid),
                        (gg, AF.Tanh), (og, AF.Sigmoid)]
                for ch in range(4):
                    pg = psgp.tile([B, 512], f32, tag="pg")
                    sl = slice(ch * 512, (ch + 1) * 512)
                    for k in range(3):
                        nc.tensor.matmul(
                            pg[:], xT[:, k, :], wih[:, k, sl],
                            start=(k == 0), stop=False,
                        )
                    for k in range(4):
                        nc.tensor.matmul(
                            pg[:], hT[:, k, :], whh[:, k, sl],
                            start=False, stop=(k == 3),
                        )
                    gt, fn = gact[ch]
                    nc.scalar.activation(gt[:], pg[:], fn)
                # ---- LSTM cell state update ----
                nc.vector.tensor_tensor(ig[:], ig[:], gg[:], op=OP.mult)
                nc.vector.tensor_tensor(cst[:], fg[:], cst[:], op=OP.mult)
                nc.vector.tensor_tensor(cst[:], cst[:], ig[:], op=OP.add)
                nc.scalar.activation(fg[:], cst[:], AF.Tanh)
                nc.vector.tensor_tensor(gg[:], og[:], fg[:], op=OP.mult)
                hh = gg  # new hidden state, batch-partition layout
                # ---- transpose h -> hT ----
                for k in range(4):
                    pt = psump.tile([128, B], f32, tag="pt")
                    nc.tensor.transpose(
                        pt[:], hh[:, k * 128:(k + 1) * 128], ident[:]
                    )
                    nc.scalar.activation(hT[:, k, :], pt[:], AF.Copy)
                # ---- vocab projection; per-chunk bias/max/exp/argmax ----
                logits = logitsp.tile([B, VSP], f32, tag="logits")
                cmax = workp.tile([B, NCH], f32, tag="cmax")
                csum = workp.tile([B, NCH], f32, tag="csum")
                cidx = workp.tile([B, NCH], f32, tag="cidx")
                for ch in range(NCH):
                    pv = psump.tile([B, 512], f32, tag="pv")
                    sl = slice(ch * 512, (ch + 1) * 512)
                    for k in range(4):
                        nc.tensor.matmul(
                            pv[:], hT[:, k, :], outw[:, k, sl],
                            start=(k == 0), stop=(k == 3),
                        )
                    # logits = pv + outb, then chunk max
                    nc.vector.tensor_tensor(
                        out=logits[:, sl], in0=pv[:], in1=outb[:, sl],
                        op=OP.add,
                    )
                    nc.vector.tensor_reduce(
                        out=cmax[:, ch:ch + 1], in_=logits[:, sl],
                        op=OP.max, axis=AX.X,
                    )
                    # sumexp of chunk (no max-shift; logits are O(1))
                    scr = chunkp.tile([B, 512], f32, tag="scr")
                    nc.scalar.activation(
                        out=scr[:], in_=logits[:, sl], func=AF.Exp,
                        accum_out=csum[:, ch:ch + 1],
                    )
                    # argmax-in-chunk: sum((logits >= cmax) * iota), one pass
                    jnk = chunkp.tile([B, 512], f32, tag="jnk")
                    nc.vector.scalar_tensor_tensor(
                        out=jnk[:], in0=logits[:, sl],
                        scalar=cmax[:, ch:ch + 1], in1=iota0[:],
                        op0=OP.is_ge, op1=OP.mult,
                        accum_out=cidx[:, ch:ch + 1],
                    )
                # ---- local stats -> [max, sumexp, globalidx] ----
                stats = workp.tile([B, 3], f32, tag="stats")
                nc.vector.tensor_reduce(
                    out=stats[:, 0:1], in_=cmax[:], op=OP.max, axis=AX.X
                )
                nc.vector.tensor_reduce(
                    out=stats[:, 1:2], in_=csum[:], op=OP.add, axis=AX.X
                )
                gidx8 = workp.tile([B, NCH], f32, tag="gidx8")
                nc.vector.tensor_tensor(
                    gidx8[:], cidx[:], chof[:], op=OP.add
                )
                jnk8 = workp.tile([B, NCH], f32, tag="jnk8")
                nc.vector.scalar_tensor_tensor(
                    out=jnk8[:], in0=cmax[:], scalar=stats[:, 0:1],
                    in1=gidx8[:], op0=OP.is_ge, op1=OP.mult,
                    accum_out=stats[:, 2:3],
                )
                # ---- AllGather stats ----
                sdram = dramp.tile([B, 3], f32, tag="sin")
                gdram = dramp.tile([NCORES * B, 3], f32, tag="gout")
                nc.gpsimd.dma_start(sdram[:], stats[:])
                nc.gpsimd.collective_compute(
                    "AllGather",
                    OP.bypass,
                    ins=[sdram[:]],
                    outs=[gdram[:]],
                    replica_groups=[list(range(NCORES))],
                )
                gath = workp.tile([B, NCORES, 3], f32, tag="gath")
                nc.gpsimd.dma_start(
                    gath[:], gdram[:].rearrange("(r b) s -> b r s", r=NCORES)
                )
                # ---- combine: logZ = ln(sum of sumexps) ----
                gsum = workp.tile([B, 1], f32, tag="gsum")
                nc.vector.tensor_reduce(
                    out=gsum[:], in_=gath[:, :, 1], op=OP.add, axis=AX.X
                )
                lngs = workp.tile([B, 1], f32, tag="lngs")
                nc.scalar.activation(lngs[:], gsum[:], AF.Ln)
                nlz = workp.tile([B, 1], f32, tag="nlz")
                nc.vector.tensor_scalar_mul(nlz[:], lngs[:], -1.0)
                # ---- logp slice -> DRAM (ACT identity with bias, in place) ----
                nc.scalar.activation(
                    out=logits[:], in_=logits[:], func=AF.Identity,
                    bias=nlz[:, 0:1],
                )
                for q in range(4):
                    nc.sync.dma_start(
                        OUT.ap()[:, t, q * 1000:(q + 1) * 1000],
                        logits[:, q * 1000:(q + 1) * 1000],
                    )
                if last:
                    continue
                # ---- global argmax index + next-x gather ----
                gmax = workp.tile([B, 1], f32, tag="gmax")
                nc.vector.tensor_reduce(
                    out=gmax[:], in_=gath[:, :, 0], op=OP.max, axis=AX.X
                )
                jnkr = workp.tile([B, NCORES], f32, tag="jnkr")
                gidx = workp.tile([B, 1], f32, tag="gidx")
                nc.vector.scalar_tensor_tensor(
                    out=jnkr[:], in0=gath[:, :, 0], scalar=gmax[:, 0:1],
                    in1=gath[:, :, 2], op0=OP.is_ge, op1=OP.mult,
                    accum_out=gidx[:],
                )
                nc.vector.tensor_scalar(
                    out=gidx[:], in0=gidx[:], scalar1=float(VQ - 1),
                    scalar2=0.0, op0=OP.min, op1=OP.max,
                )
                idxi = workp.tile([B, 1], i32, tag="idxi")
                nc.vector.tensor_copy(idxi[:], gidx[:])
                nc.gpsimd.indirect_dma_start(
                    out=xpad[:, 0:E],
                    out_offset=None,
                    in_=EMBX.ap(),
                    in_offset=bass.IndirectOffsetOnAxis(ap=idxi[:, 0:1], axis=0),
                )
                for k in range(3):
                    pt = psump.tile([128, B], f32, tag="pt")
                    nc.tensor.transpose(
                        pt[:], xpad[:, k * 128:(k + 1) * 128], ident[:]
                    )
                    nc.scalar.activation(xT[:, k, :], pt[:], AF.Copy)

    nc.finalize()
    return nc


def _prep_inputs(input_h, q_att, emb, W_ih, W_hh, b_ih, b_hh, out_W, out_b,
                 qix_to_aix):
    embx = np.maximum(
        np.asarray(emb, np.float32)[np.asarray(qix_to_aix, np.int64)], 0.0
    ).astype(np.float32)
    embx = np.ascontiguousarray(embx)
    x0 = embx[SOS]  # (300,)
    x0t = np.zeros((EP, B), np.float32)
    x0t[:E, :] = x0[:, None]
    x0t[E, :] = 1.0  # ones row driving the fused bias
    x0t = np.ascontiguousarray(x0t.reshape(3, 128, B).transpose(1, 0, 2))
    h0t = np.ascontiguousarray(
        np.asarray(q_att, np.float32).T.reshape(4, 128, B).transpose(1, 0, 2)
    )
    wih = np.zeros((EP, G), np.float32)
    wih[:E, :] = np.asarray(W_ih, np.float32).T
    wih[E, :] = np.asarray(b_ih, np.float32) + np.asarray(b_hh, np.float32)
    wih = np.ascontiguousarray(wih.reshape(3, 128, G).transpose(1, 0, 2))
    whh = np.ascontiguousarray(
        np.asarray(W_hh, np.float32).T.reshape(4, 128, G).transpose(1, 0, 2)
    )
    iota0 = np.ascontiguousarray(
        np.broadcast_to(np.arange(512, dtype=np.float32), (B, 512))
    )
    identm = np.ascontiguousarray(np.eye(B, dtype=np.float32))
    shared = dict(x0t=x0t, h0t=h0t, wih=wih, whh=whh, iota0=iota0, embx=embx,
                  identm=identm)
    in_maps = []
    for i in range(NCORES):
        sl = slice(i * VS, (i + 1) * VS)
        ow = np.zeros((H, VSP), np.float32)
        ow[:, :VS] = np.asarray(out_W, np.float32)[sl].T
        ow = np.ascontiguousarray(ow.reshape(4, 128, VSP).transpose(1, 0, 2))
        ob = np.full((VSP,), NEG_BIG, np.float32)
        ob[:VS] = np.asarray(out_b, np.float32)[sl]
        obr = np.ascontiguousarray(np.broadcast_to(ob, (B, VSP)))
        co = (i * VS + np.arange(NCH, dtype=np.float32) * 512)
        cor = np.ascontiguousarray(np.broadcast_to(co, (B, NCH)))
        m = dict(shared)
        m.update(outw=ow, outb=obr, chof=cor)
        in_maps.append(m)
    return in_maps


def kernel(input_h, q_att, emb, W_ih, W_hh, b_ih, b_hh, out_W, out_b,
           qix_to_aix, max_len, _want_results=False, _run_kwargs=None):
    from concourse import bass_utils

    T1 = int(max_len) + 1
    if T1 not in _cache:
        _cache[T1] = _build(T1)
    nc = _cache[T1]
    in_maps = _prep_inputs(input_h, q_att, emb, W_ih, W_hh, b_ih, b_hh,
                           out_W, out_b, qix_to_aix)
    res = bass_utils.run_bass_kernel_spmd(
        nc, in_maps, core_ids=list(range(NCORES)), **(_run_kwargs or {})
    )
    out = np.concatenate([res.results[i]["out"] for i in range(NCORES)],
                         axis=2)
    if _want_results:
        return out, res
    return out


# revision 10
# speedup vs baseline: 1.6403x; 1.0584x over previous
"""Greedy autoregressive LSTM decoder on 8 trn2 NeuronCores.

Strategy: vocab-shard the big projection out_W (32000x512) across 8 cores
(4000 rows each, SBUF-resident). Every core runs the full-batch (B=64) LSTM
recurrence redundantly (cheap). Each step, cores compute their local logits
slice, local (max, sumexp, argmax) stats, AllGather the tiny stats vector,
combine locally to get the global log-softmax normalizer and global argmax,
write their logp slice, and gather the next embedding row.

Host pre-fuses relu(emb[qix_to_aix]) into one (32000, 300) gather table so
the device does a single indirect DMA per step. The LSTM bias rides in the
zero-padding rows of the W_ih K-tiles (x is padded with a ones column).
sumexp is computed without max-shift: logits here are O(1) so exp cannot
overflow, and that makes the cross-core combine a plain sum.
"""

import numpy as np

B = 64
H = 512
E = 300
EP = 384  # E (+ ones row at 300) padded to 3*128
G = 2048  # 4*H
VQ = 32000
NCORES = 8
VS = VQ // NCORES  # 4000
VSP = 4096  # padded vocab slice
NCH = 8  # vocab chunks of 512
SOS = 1
NEG_BIG = -1.0e30

_cache = {}


def _build(T1):
    import concourse.bass as bass
    import concourse.bacc as bacc
    import concourse.tile as tile
    import concourse.mybir as mybir

    f32 = mybir.dt.float32
    f32r = mybir.dt.float32r
    i32 = mybir.dt.int32
    AF = mybir.ActivationFunctionType
    OP = mybir.AluOpType
    AX = mybir.AxisListType

    nc = bacc.Bacc(
        "TRN2",
        target_bir_lowering=False,
        debug=False,
        enable_asserts=False,
        num_devices=NCORES,
    )

    X0T = nc.dram_tensor("x0t", [128, 3, B], f32r, kind="ExternalInput")
    H0T = nc.dram_tensor("h0t", [128, 4, B], f32r, kind="ExternalInput")
    WIH = nc.dram_tensor("wih", [128, 3, G], f32r, kind="ExternalInput")
    WHH = nc.dram_tensor("whh", [128, 4, G], f32r, kind="ExternalInput")
    OUTW = nc.dram_tensor("outw", [128, 4, VSP], f32r, kind="ExternalInput")
    OUTB = nc.dram_tensor("outb", [B, VSP], f32, kind="ExternalInput")
    IOTA0 = nc.dram_tensor("iota0", [B, 512], f32, kind="ExternalInput")
    CHOF = nc.dram_tensor("chof", [B, NCH], f32, kind="ExternalInput")
    EMBX = nc.dram_tensor("embx", [VQ, E], f32, kind="ExternalInput")
    IDENT = nc.dram_tensor("identm", [B, B], f32, kind="ExternalInput")
    OUT = nc.dram_tensor("out", [B, T1, VS], f32, kind="ExternalOutput")

    with tile.TileContext(nc) as tc:
        with (
            tc.tile_pool(name="const", bufs=1) as constp,
            tc.tile_pool(name="state", bufs=1) as statep,
            tc.tile_pool(name="lstm", bufs=1) as lstmp,
            tc.tile_pool(name="logits", bufs=2) as logitsp,
            tc.tile_pool(name="chunk", bufs=2) as chunkp,
            tc.tile_pool(name="work", bufs=2) as workp,
            tc.tile_pool(name="psum", bufs=2, space="PSUM") as psump,
            tc.tile_pool(name="psg", bufs=4, space="PSUM") as psgp,
            tc.tile_pool(name="dram", bufs=2, space="DRAM") as dramp,
        ):
            # ---- constants (loaded once) ----
            wih = constp.tile([128, 3, G], f32r)
            whh = constp.tile([128, 4, G], f32r)
            outw = constp.tile([128, 4, VSP], f32r)
            outb = constp.tile([B, VSP], f32)
            iota0 = constp.tile([B, 512], f32)
            chof = constp.tile([B, NCH], f32)
            ident = constp.tile([B, B], f32)
            nc.sync.dma_start(wih[:], WIH.ap())
            nc.sync.dma_start(whh[:], WHH.ap())
            nc.sync.dma_start(outw[:], OUTW.ap())
            nc.sync.dma_start(outb[:], OUTB.ap())
            nc.sync.dma_start(iota0[:], IOTA0.ap())
            nc.sync.dma_start(chof[:], CHOF.ap())
            nc.sync.dma_start(ident[:], IDENT.ap())

            # ---- persistent state ----
            xT = statep.tile([128, 3, B], f32r)  # x^T (K on partitions)
            hT = statep.tile([128, 4, B], f32r)
            cst = statep.tile([B, H], f32)
            xpad = statep.tile([B, EP], f32)  # col 300 = 1.0 (bias row)
            nc.sync.dma_start(xT[:], X0T.ap())
            nc.sync.dma_start(hT[:], H0T.ap())
            nc.vector.memset(cst[:], 0.0)
            nc.vector.memset(xpad[:], 0.0)
            nc.vector.memset(xpad[:, E:E + 1], 1.0)

            for t in range(T1):
                last = t == T1 - 1
                # ---- gates = x @ W_ih.T + h @ W_hh.T + b (b in wih row E) ----
                ig = lstmp.tile([B, H], f32, tag="ig")
                fg = lstmp.tile([B, H], f32, tag="fg")
                gg = lstmp.tile([B, H], f32, tag="gg")
                og = lstmp.tile([B, H], f32, tag="og")
                gact = [(ig, AF.Sigmoid), (fg, AF.Sigmoid),
                        (gg, AF.Tanh), (og, AF.Sigmoid)]
                # W_hh part first: depends only on hT, so it can run while
                # the previous step's collective / gather are in flight.
                pgs = []
                for ch in range(4):
                    pg = psgp.tile([B, 512], f32, tag="pg")
                    pgs.append(pg)
                    sl = slice(ch * 512, (ch + 1) * 512)
                    for k in range(4):
                        nc.tensor.matmul(
                            pg[:], hT[:, k, :], whh[:, k, sl],
                            start=(k == 0), stop=False,
                        )
                for ch in range(4):
                    pg = pgs[ch]
                    sl = slice(ch * 512, (ch + 1) * 512)
                    for k in range(3):
                        nc.tensor.matmul(
                            pg[:], xT[:, k, :], wih[:, k, sl],
                            start=False, stop=(k == 2),
                        )
                    gt, fn = gact[ch]
                    nc.scalar.activation(gt[:], pg[:], fn)
                # ---- LSTM cell state update ----
                nc.vector.tensor_tensor(ig[:], ig[:], gg[:], op=OP.mult)
                nc.vector.tensor_tensor(cst[:], fg[:], cst[:], op=OP.mult)
                nc.vector.tensor_tensor(cst[:], cst[:], ig[:], op=OP.add)
                nc.scalar.activation(fg[:], cst[:], AF.Tanh)
                nc.vector.tensor_tensor(gg[:], og[:], fg[:], op=OP.mult)
                hh = gg  # new hidden state, batch-partition layout
                # ---- transpose h -> hT ----
                for k in range(4):
                    pt = psump.tile([128, B], f32, tag="pt")
                    nc.tensor.transpose(
                        pt[:], hh[:, k * 128:(k + 1) * 128], ident[:]
                    )
                    nc.scalar.activation(hT[:, k, :], pt[:], AF.Copy)
                # ---- vocab projection; per-chunk bias/max/exp/argmax ----
                logits = logitsp.tile([B, VSP], f32, tag="logits")
                cmax = workp.tile([B, NCH], f32, tag="cmax")
                csum = workp.tile([B, NCH], f32, tag="csum")
                cidx = workp.tile([B, NCH], f32, tag="cidx")
                for ch in range(NCH):
                    pv = psump.tile([B, 512], f32, tag="pv")
                    sl = slice(ch * 512, (ch + 1) * 512)
                    for k in range(4):
                        nc.tensor.matmul(
                            pv[:], hT[:, k, :], outw[:, k, sl],
                            start=(k == 0), stop=(k == 3),
                        )
                    # logits = pv + outb, then chunk max
                    nc.vector.tensor_tensor(
                        out=logits[:, sl], in0=pv[:], in1=outb[:, sl],
                        op=OP.add,
                    )
                    nc.vector.tensor_reduce(
                        out=cmax[:, ch:ch + 1], in_=logits[:, sl],
                        op=OP.max, axis=AX.X,
                    )
                    # sumexp of chunk (no max-shift; logits are O(1))
                    scr = chunkp.tile([B, 512], f32, tag="scr")
                    nc.scalar.activation(
                        out=scr[:], in_=logits[:, sl], func=AF.Exp,
                        accum_out=csum[:, ch:ch + 1],
                    )
                    # argmax-in-chunk: sum((logits >= cmax) * iota), one pass
                    jnk = chunkp.tile([B, 512], f32, tag="jnk")
                    nc.vector.scalar_tensor_tensor(
                        out=jnk[:], in0=logits[:, sl],
                        scalar=cmax[:, ch:ch + 1], in1=iota0[:],
                        op0=OP.is_ge, op1=OP.mult,
                        accum_out=cidx[:, ch:ch + 1],
                    )
                # ---- local stats -> [max, sumexp, globalidx] ----
                stats = workp.tile([B, 3], f32, tag="stats")
                nc.vector.tensor_reduce(
                    out=stats[:, 0:1], in_=cmax[:], op=OP.max, axis=AX.X
                )
                nc.vector.tensor_reduce(
                    out=stats[:, 1:2], in_=csum[:], op=OP.add, axis=AX.X
                )
                gidx8 = workp.tile([B, NCH], f32, tag="gidx8")
                nc.vector.tensor_tensor(
                    gidx8[:], cidx[:], chof[:], op=OP.add
                )
                jnk8 = workp.tile([B, NCH], f32, tag="jnk8")
                nc.vector.scalar_tensor_tensor(
                    out=jnk8[:], in0=cmax[:], scalar=stats[:, 0:1],
                    in1=gidx8[:], op0=OP.is_ge, op1=OP.mult,
                    accum_out=stats[:, 2:3],
                )
                # ---- AllGather stats ----
                sdram = dramp.tile([B, 3], f32, tag="sin")
                gdram = dramp.tile([NCORES * B, 3], f32, tag="gout")
                nc.gpsimd.dma_start(sdram[:], stats[:])
                nc.gpsimd.collective_compute(
                    "AllGather",
                    OP.bypass,
                    ins=[sdram[:]],
                    outs=[gdram[:]],
                    replica_groups=[list(range(NCORES))],
                )
                gath = workp.tile([B, NCORES, 3], f32, tag="gath")
                nc.gpsimd.dma_start(
                    gath[:], gdram[:].rearrange("(r b) s -> b r s", r=NCORES)
                )
                # ---- combine: logZ = ln(sum of sumexps) ----
                gsum = workp.tile([B, 1], f32, tag="gsum")
                nc.vector.tensor_reduce(
                    out=gsum[:], in_=gath[:, :, 1], op=OP.add, axis=AX.X
                )
                lngs = workp.tile([B, 1], f32, tag="lngs")
                nc.scalar.activation(lngs[:], gsum[:], AF.Ln)
                nlz = workp.tile([B, 1], f32, tag="nlz")
                nc.vector.tensor_scalar_mul(nlz[:], lngs[:], -1.0)
                # ---- logp slice -> DRAM (ACT identity with bias, in place) ----
                nc.scalar.activation(
                    out=logits[:], in_=logits[:], func=AF.Identity,
                    bias=nlz[:, 0:1],
                )
                for q in range(4):
                    nc.sync.dma_start(
                        OUT.ap()[:, t, q * 1000:(q + 1) * 1000],
                        logits[:, q * 1000:(q + 1) * 1000],
                    )
                if last:
                    continue
                # ---- global argmax index + next-x gather ----
                gmax = workp.tile([B, 1], f32, tag="gmax")
                nc.vector.tensor_reduce(
                    out=gmax[:], in_=gath[:, :, 0], op=OP.max, axis=AX.X
                )
                jnkr = workp.tile([B, NCORES], f32, tag="jnkr")
                gidx = workp.tile([B, 1], f32, tag="gidx")
                nc.vector.scalar_tensor_tensor(
                    out=jnkr[:], in0=gath[:, :, 0], scalar=gmax[:, 0:1],
                    in1=gath[:, :, 2], op0=OP.is_ge, op1=OP.mult,
                    accum_out=gidx[:],
                )
                nc.vector.tensor_scalar(
                    out=gidx[:], in0=gidx[:], scalar1=float(VQ - 1),
                    scalar2=0.0, op0=OP.min, op1=OP.max,
                )
                idxi = workp.tile([B, 1], i32, tag="idxi")
                nc.vector.tensor_copy(idxi[:], gidx[:])
                nc.gpsimd.indirect_dma_start(
                    out=xpad[:, 0:E],
                    out_offset=None,
                    in_=EMBX.ap(),
                    in_offset=bass.IndirectOffsetOnAxis(ap=idxi[:, 0:1], axis=0),
                )
                for k in range(3):
                    pt = psump.tile([128, B], f32, tag="pt")
                    nc.tensor.transpose(
                        pt[:], xpad[:, k * 128:(k + 1) * 128], ident[:]
                    )
                    nc.scalar.activation(xT[:, k, :], pt[:], AF.Copy)

    nc.finalize()
    return nc


def _prep_inputs(input_h, q_att, emb, W_ih, W_hh, b_ih, b_hh, out_W, out_b,
                 qix_to_aix):
    embx = np.maximum(
        np.asarray(emb, np.float32)[np.asarray(qix_to_aix, np.int64)], 0.0
    ).astype(np.float32)
    embx = np.ascontiguousarray(embx)
    x0 = embx[SOS]  # (300,)
    x0t = np.zeros((EP, B), np.float32)
    x0t[:E, :] = x0[:, None]
    x0t[E, :] = 1.0  # ones row driving the fused bias
    x0t = np.ascontiguousarray(x0t.reshape(3, 128, B).transpose(1, 0, 2))
    h0t = np.ascontiguousarray(
        np.asarray(q_att, np.float32).T.reshape(4, 128, B).transpose(1, 0, 2)
    )
    wih = np.zeros((EP, G), np.float32)
    wih[:E, :] = np.asarray(W_ih, np.float32).T
    wih[E, :] = np.asarray(b_ih, np.float32) + np.asarray(b_hh, np.float32)
    wih = np.ascontiguousarray(wih.reshape(3, 128, G).transpose(1, 0, 2))
    whh = np.ascontiguousarray(
        np.asarray(W_hh, np.float32).T.reshape(4, 128, G).transpose(1, 0, 2)
    )
    iota0 = np.ascontiguousarray(
        np.broadcast_to(np.arange(512, dtype=np.float32), (B, 512))
    )
    identm = np.ascontiguousarray(np.eye(B, dtype=np.float32))
    shared = dict(x0t=x0t, h0t=h0t, wih=wih, whh=whh, iota0=iota0, embx=embx,
                  identm=identm)
    in_maps = []
    for i in range(NCORES):
        sl = slice(i * VS, (i + 1) * VS)
        ow = np.zeros((H, VSP), np.float32)
        ow[:, :VS] = np.asarray(out_W, np.float32)[sl].T
        ow = np.ascontiguousarray(ow.reshape(4, 128, VSP).transpose(1, 0, 2))
        ob = np.full((VSP,), NEG_BIG, np.float32)
        ob[:VS] = np.asarray(out_b, np.float32)[sl]
        obr = np.ascontiguousarray(np.broadcast_to(ob, (B, VSP)))
        co = (i * VS + np.arange(NCH, dtype=np.float32) * 512)
        cor = np.ascontiguousarray(np.broadcast_to(co, (B, NCH)))
        m = dict(shared)
        m.update(outw=ow, outb=obr, chof=cor)
        in_maps.append(m)
    return in_maps


def kernel(input_h, q_att, emb, W_ih, W_hh, b_ih, b_hh, out_W, out_b,
           qix_to_aix, max_len, _want_results=False, _run_kwargs=None):
    from concourse import bass_utils

    T1 = int(max_len) + 1
    if T1 not in _cache:
        _cache[T1] = _build(T1)
    nc = _cache[T1]
    in_maps = _prep_inputs(input_h, q_att, emb, W_ih, W_hh, b_ih, b_hh,
                           out_W, out_b, qix_to_aix)
    res = bass_utils.run_bass_kernel_spmd(
        nc, in_maps, core_ids=list(range(NCORES)), **(_run_kwargs or {})
    )
    out = np.concatenate([res.results[i]["out"] for i in range(NCORES)],
                         axis=2)
    if _want_results:
        return out, res
    return out
